# revision 40
# baseline (speedup 1.0000x reference)
"""Causal self-attention (B=4, T=2048, C=768, H=12) on 8 NeuronCores.

Sharding: core <-> (batch b = core//2, heads h0 = 6*(core%2) .. h0+5).
Each core computes its 6 heads' attention for its batch element plus the
partial output projection; the host sums the two half-head partials per batch.

Device algorithm (per core):
  1. QKV^T in fp16: Q^T,K^T computed into PSUM [128 qk-cols, 512 t], evacuated
     by DVE as fp8e4 staging tiles, then shuffle-DMA'd into the DoubleRow
     layout QT8/KT8[p]: [64, 2, T] fp8 (two heads on partition halves, head
     dim split 32+32 across the free "2" slot). V lands in vaug8 (fp8, with a
     ones column for the softmax denominator) plus a dvaug8 residual
     (V - fp8(V)) so PV keeps near-fp16 accuracy at fp8 speed.
  2. Scores: S^T[tk, tq] per head via fp8 DoubleRow matmuls (32+32 contraction
     packing, 0.5 cycles/row). Q is NOT pre-scaled; the 1/sqrt(D) fold happens
     in the exp.
  3. Exp on ACT with scale=0.125 writes E directly as fp8e4 tiles [128, 2, 512]
     (two k-tiles per tile - exactly the DoubleRow rhs layout). Diagonal
     windows masked by gpsimd affine_select (which also zeroes the masked
     column prefix of odd k-tiles).
  4. PV: fp8 DoubleRow over k-tile pairs, V then dV residual, all accumulated
     in one fp32 PSUM group; the ones column yields the denominator row.
  5. Normalize: reciprocal of denom row, gpsimd partition_broadcast, DVE mul
     into fp32r O tiles.  Proj in fp32r: partial = O^T.T @ W_proj_slice.

Scheduling: pair-0's attention groups are woven into the QKV phase, later
groups are software-pipelined (scores of group g+1 before PV of group g), and
the projection is woven per-j into pair-2's attention.
"""

import numpy as np

import concourse.bass as bass
import concourse.mybir as mybir
import concourse.tile as tile
from concourse import bacc
from concourse.bass_utils import run_bass_kernel_spmd

F32 = mybir.dt.float32
F32R = mybir.dt.float32r
F16 = mybir.dt.float16
F8 = mybir.dt.float8e4
DR = mybir.MatmulPerfMode.DoubleRow

T = 2048
C = 768
D = 64
HPC = 6          # heads per core
NCC = 6          # C / 128
NT = 16          # T / 128
NJ = 4           # T / 512
EXP = mybir.ActivationFunctionType.Exp


def _emit(nc, tc, xT, wqk, wv, wp, out):
    from contextlib import ExitStack
    with ExitStack() as ctx:
        pp = ctx.enter_context(tc.tile_pool(name="persist", bufs=1))

        # DoubleRow Q/K: per pair p, [64, 2*T] fp8; partitions 0:32 = head 2p,
        # 32:64 = head 2p+1; free = (d-half slot, t).
        qt8 = [pp.tile([64, 2 * T], F8, tag=f"qt{p}", name=f"qt8_{p}") for p in range(3)]
        kt8 = [pp.tile([64, 2 * T], F8, tag=f"kt{p}", name=f"kt8_{p}") for p in range(3)]
        # V (fp8, ones col) and residual dV per k-tile pair: [128, 2*VSL].
        # Slot stride padded 390 -> 400: DoubleRow ldweights requires the
        # outer free-AP step to be 16-aligned.
        VSL = 400
        vaug = [pp.tile([128, 2 * VSL], F8, tag=f"v{t}", name=f"vaug{t}")
                for t in range(NT // 2)]
        dvaug = [pp.tile([128, 2 * VSL], F8, tag=f"dv{t}", name=f"dvaug{t}")
                 for t in range(NT // 2)]
        OF = [pp.tile([128, T], F32R, tag=f"of{p}", name=f"of{p}") for p in range(3)]
        wp_t = [pp.tile([128, C], F32R, tag=f"wp{p}", name=f"wp{p}") for p in range(3)]

        # attention pools (open for the whole kernel; PSUM: S 4 banks + O 2)
        epl = ctx.enter_context(tc.tile_pool(name="epool", bufs=28))
        attsm = ctx.enter_context(tc.tile_pool(name="attsm", bufs=1))
        osb = ctx.enter_context(tc.tile_pool(name="outsb", bufs=4))
        sp = ctx.enter_context(tc.tile_pool(name="spsum", bufs=2, space="PSUM"))
        op_ = ctx.enter_context(tc.tile_pool(name="opsum", bufs=1, space="PSUM"))

        warm = attsm.tile([1, 8], F32, tag="warm", name="warmup")
        nc.vector.memset(warm[:], 0.0)
        nc.scalar.activation(warm[0:1, 0:8], warm[0:1, 0:8], EXP)

        otmp = {}
        pending_proj = []

        def gen_ph1(p, j, es):
            """scores + exp for group (p, j); fills E slots dict es."""
            if p not in otmp:
                otmp[p] = attsm.tile([64, T], F32R, tag="otmp", name=f"otmp{p}")
            QTv = qt8[p].rearrange("p (two t) -> p two t", two=2)
            KTv = kt8[p].rearrange("p (two t) -> p two t", two=2)
            ni = 4 * j + 4
            for k in range(ni // 2):
                if k == max(1, ni // 4):
                    yield
                i0 = 2 * k
                poff = max(0, 256 * k - 512 * j)
                ss = {}
                for sub in (0, 1):
                    ss[sub] = sp.tile([128, 1024], F32, tag="s", name=f"s{p}{j}{k}{sub}")
                for idx in (0, 1):
                    i = i0 + idx
                    isl = slice(128 * i, 128 * (i + 1))
                    for sub in (0, 1):
                        b0 = 32 * sub
                        nc.tensor.matmul(
                            out=ss[sub][:, 512 * idx + poff:512 * idx + 512],
                            lhsT=KTv[b0:b0 + 32, :, isl],
                            rhs=QTv[b0:b0 + 32, :, 512 * j + poff:512 * (j + 1)],
                            start=True, stop=True, perf_mode=DR,
                        )
                for sub in (0, 1):
                    e = epl.tile([128, 1024], F8, tag="e", name=f"e{p}{j}{k}{sub}")
                    if poff:
                        e3 = e.rearrange("p (two t) -> p two t", two=2)
                        s3 = ss[sub].rearrange("p (two t) -> p two t", two=2)
                        nc.scalar.activation(e3[:, :, poff:512], s3[:, :, poff:512],
                                             EXP, scale=0.125)
                    else:
                        nc.scalar.activation(e[:], ss[sub][:], EXP, scale=0.125)
                    for idx in (0, 1):
                        i = i0 + idx
                        if i >= 4 * j:
                            vi = 128 * i - 512 * j
                            win = e[:, 512 * idx + poff:512 * idx + vi + 128]
                            # keep iff (global col - vi) >= row
                            nc.gpsimd.affine_select(
                                out=win, in_=win,
                                pattern=[[1, vi + 128 - poff]],
                                compare_op=mybir.AluOpType.is_ge,
                                fill=0.0, base=poff - vi, channel_multiplier=-1,
                            )
                    es[(sub, k)] = e

        def gen_ph2(p, j, es, opool=None, raw=False):
            """PV accumulate + normalize/evacuate for group (p, j)."""
            opool = opool or op_
            npairs = 2 * j + 2
            jsl = slice(512 * j, 512 * (j + 1))
            ot = {sub: opool.tile([D + 1, 512], F32, tag=f"o{sub}", name=f"ot{p}{j}{sub}")
                  for sub in (0, 1)}
            for kk in range(npairs):
                if kk == max(1, npairs // 2):
                    yield
                poff = max(0, 256 * kk - 512 * j)
                for sub in (0, 1):
                    h = 2 * p + sub
                    e3 = es[(sub, kk)].rearrange("p (two t) -> p two t", two=2)
                    rhs = e3[:, :, poff:512]
                    for vt, first in ((vaug, True), (dvaug, False)):
                        v3 = vt[kk].rearrange("p (two hc) -> p two hc", two=2)
                        nc.tensor.matmul(
                            out=ot[sub][:, poff:512],
                            lhsT=v3[:, :, 65 * h:65 * h + 65],
                            rhs=rhs,
                            start=(kk == 0 and first),
                            stop=(kk == npairs - 1 and not first),
                            perf_mode=DR,
                        )
            for sub in (0, 1):
                src_t = ot[sub]
                if raw:
                    oc = attsm.tile([D + 1, 512], F32, tag=f"oc{sub}", bufs=2,
                                    name=f"oc{p}{j}{sub}")
                    nc.vector.tensor_copy(oc[:], ot[sub][:])
                    src_t = oc
                r_ = attsm.tile([1, 512], F32, tag=f"r{sub}", name=f"r{p}{j}{sub}")
                nc.vector.reciprocal(r_[0:1, :], src_t[D:D + 1, :])
                rb = attsm.tile([64, 512], F32, tag=f"rb{sub}", name=f"rb{p}{j}{sub}")
                nc.gpsimd.partition_broadcast(rb[0:64, :], r_[0:1, :])
                dst = OF[p][0:64, jsl] if sub == 0 else otmp[p][0:64, jsl]
                nc.vector.tensor_mul(dst, src_t[0:D, :], rb[0:64, :])
            nc.gpsimd.dma_start(out=OF[p][64:128, jsl], in_=otmp[p][0:64, jsl])
            if p == 2:
                pending_proj.append(j)

        def emit_proj(j, ppools):
            slots = [(pl, f"o{s}") for pl in ppools for s in (0, 1)]
            si = 0
            for t in range(4 * j, 4 * j + 4):
                ob = osb.tile([128, C], F32, tag="ob", name=f"ob{t}")
                for eo, el in ((0, 512), (512, 256)):
                    pl, tg = slots[si % len(slots)]
                    si += 1
                    ps = pl.tile([128, el], F32, tag=tg, name=f"pps{t}_{eo}")
                    for p in range(3):
                        nc.tensor.matmul(
                            out=ps[:],
                            lhsT=OF[p][:, 128 * t:128 * (t + 1)],
                            rhs=wp_t[p][:, eo:eo + el],
                            start=(p == 0), stop=(p == 2),
                        )
                    nc.vector.tensor_copy(ob[:, eo:eo + el], ps[:])
                    nc.sync.dma_start(out=out[128 * t:128 * (t + 1), eo:eo + el],
                                      in_=ob[:, eo:eo + el])

        # ---------------- QKV phase, woven with pair-0 attention ----------
        qkv_ctx = ExitStack()
        xtw = qkv_ctx.enter_context(tc.tile_pool(name="xtw", bufs=1))
        qps = qkv_ctx.enter_context(tc.tile_pool(name="qkvps", bufs=2, space="PSUM"))
        xt_t = [[xtw.tile([128, 1024], F16, tag=f"xt{c}_{q}", name=f"xt{c}_{q}")
                 for q in range(2)] for c in range(NCC)]
        wqk_t = [xtw.tile([128, C], F16, tag=f"wqk{c}", name=f"wqkt{c}") for c in range(NCC)]
        wv_t = [xtw.tile([128, HPC * D], F16, tag=f"wv{c}", name=f"wvt{c}") for c in range(NCC)]
        qk8 = [xtw.tile([128, T], F8, tag=f"qk8_{m}", name=f"qk8_{m}") for m in range(6)]

        def dma_xt_half(q):
            for c in range(NCC):
                nc.sync.dma_start(
                    out=xt_t[c][q][:],
                    in_=xT[128 * c:128 * (c + 1), 1024 * q:1024 * (q + 1)])

        dma_xt_half(0)
        for c in range(NCC):
            nc.sync.dma_start(out=wqk_t[c][:], in_=wqk[128 * c:128 * (c + 1), :])
        dma_xt_half(1)
        for c in range(NCC):
            nc.sync.dma_start(out=wv_t[c][:], in_=wv[128 * c:128 * (c + 1), :])
        for p in range(3):
            nc.sync.dma_start(out=wp_t[p][:],
                              in_=wp[128 * p:128 * (p + 1), :].bitcast(F32R))

        def emit_qk(m, j):
            """QK^T quarter (fp16 matmul) -> fp8 staging -> shuffle-DMA into
            the DoubleRow layout of qt8/kt8."""
            ps = qps.tile([128, 512], F32, tag="qkv", name=f"qkps{m}_{j}")
            for c in range(NCC):
                nc.tensor.matmul(
                    out=ps[:],
                    lhsT=wqk_t[c][:, 128 * m:128 * (m + 1)],
                    rhs=xt_t[c][j // 2][:, 512 * (j % 2):512 * (j % 2 + 1)],
                    start=(c == 0), stop=(c == NCC - 1),
                )
            jsl = slice(512 * j, 512 * (j + 1))
            nc.vector.tensor_copy(qk8[m][:, jsl], ps[:])
            t8 = qt8[m] if m < 3 else kt8[m - 3]
            t8v = t8.rearrange("p (two t) -> p two t", two=2)
            for g in range(4):
                hb, slot = g // 2, g % 2
                nc.sync.dma_start(
                    out=t8v[32 * hb:32 * hb + 32, slot:slot + 1, jsl],
                    in_=qk8[m][32 * g:32 * g + 32, jsl])

        def emit_v(t):
            kk, slot = t // 2, t % 2
            if slot == 0:
                nc.vector.memset(vaug[kk][:], 1.0)
                nc.vector.memset(dvaug[kk][:], 0.0)
            ps = qps.tile([128, 512], F32, tag="qkv", name=f"vps{t}")[:, 0:HPC * D]
            for c in range(NCC):
                nc.tensor.matmul(
                    out=ps[:],
                    lhsT=xt_t[c][t // 8][:, 128 * (t % 8):128 * (t % 8 + 1)],
                    rhs=wv_t[c][:],
                    start=(c == 0), stop=(c == NCC - 1),
                )
            base = slot * 400
            vd = vaug[kk][:, base:base + HPC * (D + 1)]
            dvd = dvaug[kk][:, base:base + HPC * (D + 1)]
            dst = vd.rearrange("p (h c) -> p h c", c=D + 1)[:, :, 0:D]
            ddst = dvd.rearrange("p (h c) -> p h c", c=D + 1)[:, :, 0:D]
            src = ps.rearrange("p (h c) -> p h c", c=D)
            nc.vector.tensor_copy(dst, src)
            nc.vector.tensor_sub(ddst, src, dst)

        E = {}

        def drain(g):
            for _ in g:
                pass

        def step(g):
            next(g, None)

        # ---- weave: pair-0 attention + QKV interleaved ----
        for j in range(NJ):
            emit_qk(0, j); emit_qk(3, j)
            E[j] = {}
            drain(gen_ph1(0, j, E[j]))
            emit_qk(1, j); emit_qk(4, j)
            emit_qk(2, j); emit_qk(5, j)
            if j == 1:
                for t in range(0, 4):
                    emit_v(t)
                drain(gen_ph2(0, 0, E[0], raw=True))
            elif j == 2:
                for t in range(4, 8):
                    emit_v(t)
                drain(gen_ph2(0, 1, E[1], raw=True))
            elif j == 3:
                for t in range(8, 16):
                    emit_v(t)
                drain(gen_ph2(0, 2, E[2], raw=True))
        qkv_ctx.close()   # free xtw SBUF + qkv PSUM banks

        # freed qkv PSUM banks: second O pool (double-buffers PV groups)
        opB = ctx.enter_context(tc.tile_pool(name="opsumB", bufs=1, space="PSUM"))

        # ---------------- pairs 1-2, software-pipelined -------------------
        prev = (0, 3, E[3])
        pools = [op_, opB]
        for gi, (p, j) in enumerate([(1, 0), (1, 1), (1, 2), (1, 3),
                                     (2, 1), (2, 2), (2, 3), (2, 0)]):
            es = {}
            drain(gen_ph1(p, j, es))
            while pending_proj:
                emit_proj(pending_proj.pop(0), [pools[(gi + 1) % 2]])
            drain(gen_ph2(prev[0], prev[1], prev[2], pools[gi % 2],
                          raw=(prev[0] == 0)))
            prev = (p, j, es)
        # tail: prev == (2, 0); pending == [3] from ph2(2,3)
        emit_proj(pending_proj.pop(0), [pools[0]])
        drain(gen_ph2(prev[0], prev[1], prev[2], pools[1], raw=True))
        emit_proj(pending_proj.pop(0), [pools[0], pools[1]])


_NC_CACHE = None


def build_nc():
    global _NC_CACHE
    if _NC_CACHE is not None:
        return _NC_CACHE
    nc = bacc.Bacc(trn_type="TRN2")
    xT = nc.dram_tensor("xT", [C, T], F16, kind="ExternalInput").ap()
    wqk = nc.dram_tensor("wqk", [C, C], F16, kind="ExternalInput").ap()
    wv = nc.dram_tensor("wv", [C, HPC * D], F16, kind="ExternalInput").ap()
    wp = nc.dram_tensor("wp", [HPC * D, C], F32, kind="ExternalInput").ap()
    out = nc.dram_tensor("out", [T, C], F32, kind="ExternalOutput").ap()
    with tile.TileContext(nc) as tc:
        _emit(nc, tc, xT, wqk, wv, wp, out)
    nc.compile()
    _NC_CACHE = nc
    return nc


def make_in_maps(x, W_attn, W_proj):
    x = np.asarray(x, dtype=np.float32)
    W_attn = np.asarray(W_attn, dtype=np.float32)
    W_proj = np.asarray(W_proj, dtype=np.float32)
    in_maps = []
    for core in range(8):
        b = core // 2
        h0 = HPC * (core % 2)
        xT = np.ascontiguousarray(x[b].T).astype(np.float16)
        q_cols = W_attn[:, 64 * h0:64 * h0 + 384]
        k_cols = W_attn[:, 768 + 64 * h0:768 + 64 * h0 + 384]
        wqk = np.concatenate([q_cols, k_cols], axis=1).astype(np.float16)
        wv = W_attn[:, 1536 + 64 * h0:1536 + 64 * h0 + 384].astype(np.float16)
        wp = np.ascontiguousarray(W_proj[64 * h0:64 * h0 + 384, :])
        in_maps.append({"xT": xT, "wqk": np.ascontiguousarray(wqk),
                        "wv": wv, "wp": wp})
    return in_maps


def kernel(x, W_attn, W_proj, _trace=False, _trace_kwargs=None):
    nc = build_nc()
    in_maps = make_in_maps(x, W_attn, W_proj)
    res = run_bass_kernel_spmd(nc, in_maps, list(range(8)), trace=_trace,
                               **(_trace_kwargs or {}))
    outs = [res.results[c]["out"] for c in range(8)]
    y = np.stack([outs[2 * b] + outs[2 * b + 1] for b in range(4)]).astype(np.float32)
    if _trace:
        return y, res
    return y


# revision 41
# speedup vs baseline: 1.0894x; 1.0894x over previous
"""Causal self-attention (B=4, T=2048, C=768, H=12) on 8 NeuronCores.

Sharding: core <-> (batch b = core//2, heads h0 = 6*(core%2) .. h0+5).
Each core computes its 6 heads' attention for its batch element plus the
partial output projection; the host sums the two half-head partials per batch.

Device algorithm (per core):
  1. QKV^T in fp16: per (pair, T-quarter) Q^T and K^T go PSUM -> fp8 stage tile
     (DVE) -> 4 shuffle DMAs into qkt8[pair]: [64, slot2, qk2, T] fp8 - the
     DoubleRow layout (two heads on partition halves, head dim split 32+32
     across the slot dim). V lands in vaug (fp8, ones column for the softmax
     denominator) plus a dvaug residual (V - fp8(V)) for accuracy.
  2. Scores: S^T[tk, tq] per head via fp8 DoubleRow matmuls (0.5 cycles/row).
     Q is not pre-scaled; 1/sqrt(D) folds into the exp scale.
  3. Exp on ACT (scale=0.125) writes E directly as fp8 tiles [128, 2, 512]
     (two k-tiles per tile == the DoubleRow rhs layout). Diagonal windows
     masked by one gpsimd affine_select per (sub, pair) spanning both k-tiles.
  4. PV: fp8 DoubleRow over k-tile pairs, V then dV residual, accumulated in
     one fp32 PSUM group; the ones column yields the denominator row.
  5. Normalize: O evacuated to SBUF, reciprocal of denom row, gpsimd
     partition_broadcast, DVE mul into fp32r O tiles. Proj in fp32r.

Scheduling is quarter-major so ACT (the bottleneck engine) is fed from ~10us:
for each T-quarter j, all three head-pairs' scores+exp run back to back, with
QKV matmuls, V, the previous group's PV and the previous quarter's projection
woven between them. PSUM: S 2x2 banks + QKV 1 + O 2 + proj 1 = 8 banks.
"""

import numpy as np

import concourse.bass as bass
import concourse.mybir as mybir
import concourse.tile as tile
from concourse import bacc
from concourse.bass_utils import run_bass_kernel_spmd

F32 = mybir.dt.float32
F32R = mybir.dt.float32r
F16 = mybir.dt.float16
F8 = mybir.dt.float8e4
DR = mybir.MatmulPerfMode.DoubleRow

T = 2048
C = 768
D = 64
HPC = 6          # heads per core
NCC = 6          # C / 128
NT = 16          # T / 128
NJ = 4           # T / 512
VSL = 400        # vaug slot stride (390 padded to 16-align for DoubleRow)
EXP = mybir.ActivationFunctionType.Exp


def _emit(nc, tc, xT, wqk, wv, wp, out):
    from contextlib import ExitStack
    with ExitStack() as ctx:
        pp = ctx.enter_context(tc.tile_pool(name="persist", bufs=1))

        # DoubleRow Q/K per pair: [64, slot2 * qk2 * T] fp8; partitions 0:32 =
        # head 2p, 32:64 = head 2p+1; free = (d-half slot, q/k, t).
        qkt8 = [pp.tile([64, 4 * T], F8, tag=f"qkt{p}", name=f"qkt8_{p}")
                for p in range(3)]
        vaug = [pp.tile([128, 2 * VSL], F8, tag=f"v{t}", name=f"vaug{t}")
                for t in range(NT // 2)]
        dvaug = [pp.tile([128, 2 * VSL], F8, tag=f"dv{t}", name=f"dvaug{t}")
                 for t in range(NT // 2)]
        OF = [pp.tile([128, T], F32R, tag=f"of{p}", name=f"of{p}") for p in range(3)]
        wp_t = [pp.tile([128, C], F32R, tag=f"wp{p}", name=f"wp{p}") for p in range(3)]

        epl = ctx.enter_context(tc.tile_pool(name="epool", bufs=52))
        attsm = ctx.enter_context(tc.tile_pool(name="attsm", bufs=1))
        osb = ctx.enter_context(tc.tile_pool(name="outsb", bufs=4))
        stg = ctx.enter_context(tc.tile_pool(name="stage", bufs=3))
        xtw = ctx.enter_context(tc.tile_pool(name="xtw", bufs=1))
        sp = ctx.enter_context(tc.tile_pool(name="spsum", bufs=2, space="PSUM"))
        qps = ctx.enter_context(tc.tile_pool(name="qkvps", bufs=2, space="PSUM"))
        op_ = ctx.enter_context(tc.tile_pool(name="opsum", bufs=1, space="PSUM"))

        xt_t = [[xtw.tile([128, 1024], F16, tag=f"xt{c}_{q}", name=f"xt{c}_{q}")
                 for q in range(2)] for c in range(NCC)]
        wqk_t = [xtw.tile([128, C], F16, tag=f"wqk{c}", name=f"wqkt{c}")
                 for c in range(NCC)]
        wv_t = [xtw.tile([128, HPC * D], F16, tag=f"wv{c}", name=f"wvt{c}")
                for c in range(NCC)]

        warm = attsm.tile([1, 8], F32, tag="warm", name="warmup")
        nc.vector.memset(warm[:], 0.0)
        nc.scalar.activation(warm[0:1, 0:8], warm[0:1, 0:8], EXP)


        def gen_ph1(p, j, es):
            """scores + exp for group (p, j); fills E slots dict es."""
            kv = qkt8[p].rearrange("p (two qk t) -> p two qk t", two=2, qk=2)
            ni = 4 * j + 4
            for k in range(ni // 2):
                i0 = 2 * k
                poff = max(0, 256 * k - 512 * j)
                ss = {}
                for sub in (0, 1):
                    ss[sub] = sp.tile([128, 1024], F32, tag="s", name=f"s{p}{j}{k}{sub}")
                for sub in (0, 1):
                    b0 = 32 * sub
                    for idx in (0, 1):
                        i = i0 + idx
                        isl = slice(128 * i, 128 * (i + 1))
                        nc.tensor.matmul(
                            out=ss[sub][:, 512 * idx + poff:512 * idx + 512],
                            lhsT=kv[b0:b0 + 32, :, 1:2, isl],
                            rhs=kv[b0:b0 + 32, :, 0:1, 512 * j + poff:512 * (j + 1)],
                            start=True, stop=True, perf_mode=DR,
                        )
                for sub in (0, 1):
                    e = epl.tile([128, 1024], F8, tag="e", name=f"e{p}{j}{k}{sub}")
                    if poff:
                        e3 = e.rearrange("p (two t) -> p two t", two=2)
                        s3 = ss[sub].rearrange("p (two t) -> p two t", two=2)
                        nc.scalar.activation(e3[:, :, poff:512], s3[:, :, poff:512],
                                             EXP, scale=0.125)
                    else:
                        nc.scalar.activation(e[:], ss[sub][:], EXP, scale=0.125)
                    if i0 + 1 >= 4 * j:
                        # one affine_select spanning both k-tile slots:
                        # keep iff col - 128*idx - (vi0 - poff) >= row
                        vi0 = max(0, 128 * i0 - 512 * j)
                        w = vi0 + 256 - poff
                        e3 = e.rearrange("p (two t) -> p two t", two=2)
                        win = e3[:, :, poff:poff + w]
                        nc.gpsimd.affine_select(
                            out=win, in_=win,
                            pattern=[[-128, 2], [1, w]],
                            compare_op=mybir.AluOpType.is_ge,
                            fill=0.0, base=poff - vi0, channel_multiplier=-1,
                        )
                    es[(sub, k)] = e
                yield

        def gen_ph2(p, j, es):
            """PV accumulate + normalize/evacuate for group (p, j)."""
            npairs = 2 * j + 2
            jsl = slice(512 * j, 512 * (j + 1))
            ot = {sub: op_.tile([D + 1, 512], F32, tag=f"o{sub}", name=f"ot{p}{j}{sub}")
                  for sub in (0, 1)}
            for kk in range(npairs):
                poff = max(0, 256 * kk - 512 * j)
                for sub in (0, 1):
                    h = 2 * p + sub
                    e3 = es[(sub, kk)].rearrange("p (two t) -> p two t", two=2)
                    rhs = e3[:, :, poff:512]
                    for vt, first in ((vaug, True), (dvaug, False)):
                        v3 = vt[kk].rearrange("p (two hc) -> p two hc", two=2)
                        nc.tensor.matmul(
                            out=ot[sub][:, poff:512],
                            lhsT=v3[:, :, 65 * h:65 * h + 65],
                            rhs=rhs,
                            start=(kk == 0 and first),
                            stop=(kk == npairs - 1 and not first),
                            perf_mode=DR,
                        )
                yield
            # sub1 first: its result needs the extra otmp->OF partition move,
            # so get that DMA going while sub0's normalize still runs
            otm = attsm.tile([64, 512], F32R, tag="otmp", bufs=3,
                             name=f"otm{p}{j}")
            for sub in (1, 0):
                oc = attsm.tile([D + 1, 512], F32, tag=f"oc{sub}", bufs=2,
                                name=f"oc{p}{j}{sub}")
                nc.vector.tensor_copy(oc[:], ot[sub][:])
                r_ = attsm.tile([1, 512], F32, tag=f"r{sub}", bufs=2,
                                name=f"r{p}{j}{sub}")
                nc.vector.reciprocal(r_[0:1, :], oc[D:D + 1, :])
                rb = attsm.tile([64, 512], F32, tag=f"rb{sub}", bufs=2,
                                name=f"rb{p}{j}{sub}")
                nc.gpsimd.partition_broadcast(rb[0:64, :], r_[0:1, :])
                dst = OF[p][0:64, jsl] if sub == 0 else otm[0:64, :]
                nc.vector.tensor_mul(dst, oc[0:D, :], rb[0:64, :])
                if sub == 1:
                    nc.gpsimd.dma_start(out=OF[p][64:128, jsl],
                                        in_=otm[0:64, :])

        def emit_proj(j):
            for t in range(4 * j, 4 * j + 4):
                ob = osb.tile([128, C], F32, tag="ob", name=f"ob{t}")
                for eo, el in ((0, 512), (512, 256)):
                    ps = qps.tile([128, 512], F32, tag="qkv", name=f"pps{t}_{eo}")
                    for p in range(3):
                        nc.tensor.matmul(
                            out=ps[:, 0:el],
                            lhsT=OF[p][:, 128 * t:128 * (t + 1)],
                            rhs=wp_t[p][:, eo:eo + el],
                            start=(p == 0), stop=(p == 2),
                        )
                    nc.vector.tensor_copy(ob[:, eo:eo + el], ps[:, 0:el])
                nc.sync.dma_start(out=out[128 * t:128 * (t + 1), :], in_=ob[:])

        def emit_proj_final(j):
            """Final projection quarter: open all 8 accumulation chains
            across the now-idle PSUM tags, emit the p=0,1 matmuls up front
            (they only need OF quarters already normalized), and close with
            p=2 once the final group's OF slice lands - keeps PE warm through
            the last normalize instead of going cold."""
            chains = []
            for ti, t in enumerate(range(4 * j, 4 * j + 4)):
                if ti < 2:
                    ps = sp.tile([128, 1024], F32, tag="s", name=f"fpj{t}")
                    chains.append((ps[:, 0:512], t, 0, 512))
                    chains.append((ps[:, 512:768], t, 512, 256))
                else:
                    ps = op_.tile([128, 512], F32, tag=f"o{ti - 2}", name=f"fpj{t}")
                    chains.append((ps[:, 0:512], t, 0, 512))
                    # separate tiles: two chains in one tile would share a
                    # 2KB zero-region and start=True would wipe the other
                    qt = qps.tile([128, 512], F32, tag="qkv", name=f"fpjq{t}")
                    chains.append((qt[:, 0:256], t, 512, 256))
            for p in range(2):
                for ap, t, eo, el in chains:
                    nc.tensor.matmul(out=ap, lhsT=OF[p][:, 128 * t:128 * (t + 1)],
                                     rhs=wp_t[p][:, eo:eo + el],
                                     start=(p == 0), stop=False)
            obs = {}
            for ap, t, eo, el in chains:
                nc.tensor.matmul(out=ap, lhsT=OF[2][:, 128 * t:128 * (t + 1)],
                                 rhs=wp_t[2][:, eo:eo + el], start=False, stop=True)
                if t not in obs:
                    obs[t] = osb.tile([128, C], F32, tag="ob", name=f"fob{t}")
                nc.vector.tensor_copy(obs[t][:, eo:eo + el], ap)
            for t in range(4 * j, 4 * j + 4):
                nc.sync.dma_start(out=out[128 * t:128 * (t + 1), :], in_=obs[t][:])

        def emit_qk2(p, j):
            """Q^T,K^T quarter for pair p -> fp8 stage -> shuffle into qkt8."""
            st = stg.tile([128, 1024], F8, tag="st", name=f"st{p}{j}")
            for which, m in ((0, p), (1, 3 + p)):
                ps = qps.tile([128, 512], F32, tag="qkv", name=f"qkps{m}_{j}")
                for c in range(NCC):
                    nc.tensor.matmul(
                        out=ps[:],
                        lhsT=wqk_t[c][:, 128 * m:128 * (m + 1)],
                        rhs=xt_t[c][j // 2][:, 512 * (j % 2):512 * (j % 2 + 1)],
                        start=(c == 0), stop=(c == NCC - 1),
                    )
                nc.vector.tensor_copy(st[:, 512 * which:512 * (which + 1)], ps[:])
            kv = qkt8[p].rearrange("p (two qk t) -> p two qk t", two=2, qk=2)
            sv = st.rearrange("p (qk t) -> p qk t", qk=2)
            jsl = slice(512 * j, 512 * (j + 1))
            for g in range(4):
                hb, slot = g // 2, g % 2
                nc.sync.dma_start(
                    out=kv[32 * hb:32 * hb + 32, slot:slot + 1, :, jsl],
                    in_=sv[32 * g:32 * g + 32, :, :])

        def emit_v(t):
            kk, slot = t // 2, t % 2
            base = slot * VSL
            # only the ones-columns need init; V copies cover the rest and
            # the 390..400 pad is never read by the matmuls
            ones = vaug[kk][:, base:base + HPC * (D + 1)].rearrange(
                "p (h c) -> p h c", c=D + 1)[:, :, D:D + 1]
            zcol = dvaug[kk][:, base:base + HPC * (D + 1)].rearrange(
                "p (h c) -> p h c", c=D + 1)[:, :, D:D + 1]
            nc.vector.memset(ones, 1.0)
            nc.vector.memset(zcol, 0.0)
            ps = qps.tile([128, 512], F32, tag="qkv", name=f"vps{t}")[:, 0:HPC * D]
            for c in range(NCC):
                nc.tensor.matmul(
                    out=ps[:],
                    lhsT=xt_t[c][t // 8][:, 128 * (t % 8):128 * (t % 8 + 1)],
                    rhs=wv_t[c][:],
                    start=(c == 0), stop=(c == NCC - 1),
                )
            vd = vaug[kk][:, base:base + HPC * (D + 1)]
            dvd = dvaug[kk][:, base:base + HPC * (D + 1)]
            dst = vd.rearrange("p (h c) -> p h c", c=D + 1)[:, :, 0:D]
            ddst = dvd.rearrange("p (h c) -> p h c", c=D + 1)[:, :, 0:D]
            src = ps.rearrange("p (h c) -> p h c", c=D)
            nc.vector.tensor_copy(dst, src)
            nc.vector.tensor_sub(ddst, src, dst)

        _DONE = object()

        def drain(g):
            if g is not None:
                for _ in g:
                    pass

        def steps(g, n):
            if g is None:
                return False
            for _ in range(n):
                if next(g, _DONE) is _DONE:
                    return False
            return True

        # -------- input DMAs: interleave wqk/xt-half0 so QK starts early ----
        for c in range(NCC):
            nc.sync.dma_start(out=wqk_t[c][:], in_=wqk[128 * c:128 * (c + 1), :])
            nc.sync.dma_start(out=xt_t[c][0][:], in_=xT[128 * c:128 * (c + 1), 0:1024])

        # -------- schedule -------------------------------------------------
        # qk2 (QK matmul -> stage evac -> shuffle DMA) is emitted from its
        # own K-prefix-ordered list, at most two chains per slot and about a
        # slot ahead of use, so its latency hides under earlier exps. The
        # group order runs the big quarters mid-stream and finishes on the
        # three tiny j=0 groups, so the post-last-exp tail is one small PV +
        # normalize + the split final projection of quarter 0.
        groups = [(0, 0), (1, 0), (0, 1), (1, 1), (2, 1), (0, 2),
                  (1, 2), (2, 2), (2, 3), (0, 3), (1, 3), (2, 0)]
        qklist = [(0, 0), (1, 0), (2, 0), (0, 1), (1, 1), (2, 1),
                  (0, 2), (1, 2), (2, 2), (2, 3), (0, 3), (1, 3)]

        def prefetch(qi):
            if qi < len(qklist):
                emit_qk2(*qklist[qi])

        prefetch(0)
        prefetch(1)
        for c in range(NCC):   # wv behind the first shuffles in the queue
            nc.sync.dma_start(out=wv_t[c][:], in_=wv[128 * c:128 * (c + 1), :])
        prev = None        # (p, j, es) whose PV is pending
        g2 = None          # the in-flight PV generator
        g2grp = None       # the group g2 belongs to
        for gi, (p, j) in enumerate(groups):
                if p == 0:
                    if j == 1:   # deferred inputs, off the critical path
                        for c in range(NCC):
                            nc.sync.dma_start(
                                out=xt_t[c][1][:],
                                in_=xT[128 * c:128 * (c + 1), 1024:2048])
                        for q in range(3):
                            nc.sync.dma_start(
                                out=wp_t[q][:],
                                in_=wp[128 * q:128 * (q + 1), :].bitcast(F32R))
                    emit_v(4 * j); emit_v(4 * j + 1)
                if p == 1:
                    emit_v(4 * j + 2); emit_v(4 * j + 3)
                es = {}
                g1 = gen_ph1(p, j, es)
                steps(g1, 1)
                prefetch(gi + 2)
                steps(g1, 1)
                if gi == len(groups) - 1:
                    # tail: drain the two pending PV groups, then run our own
                    # small PV one pair behind our exps
                    while steps(g2, 1):
                        steps(g1, 1)
                    drain(g2)
                    if g2grp and g2grp[0] == 2:
                        emit_proj(g2grp[1])
                    gprev = gen_ph2(*prev)
                    gself = gen_ph2(p, j, es)
                    steps(gself, 1)
                    while steps(gprev, 1):
                        steps(g1, 1)
                        steps(gself, 1)
                    drain(gprev)
                    emit_proj(3)
                    drain(g1)
                    drain(gself)
                    emit_proj_final(j)
                else:
                    # interleave remaining scores with the pending group's PV
                    while steps(g1, 1):
                        steps(g2, 1)
                    drain(g2)
                    if g2grp and g2grp[0] == 2 and g2grp[1] < 3:
                        emit_proj(g2grp[1])
                    g2 = gen_ph2(*prev) if prev else None
                    g2grp = prev
                    prev = (p, j, es)


_NC_CACHE = None


def build_nc():
    global _NC_CACHE
    if _NC_CACHE is not None:
        return _NC_CACHE
    nc = bacc.Bacc(trn_type="TRN2")
    xT = nc.dram_tensor("xT", [C, T], F16, kind="ExternalInput").ap()
    wqk = nc.dram_tensor("wqk", [C, C], F16, kind="ExternalInput").ap()
    wv = nc.dram_tensor("wv", [C, HPC * D], F16, kind="ExternalInput").ap()
    wp = nc.dram_tensor("wp", [HPC * D, C], F32, kind="ExternalInput").ap()
    out = nc.dram_tensor("out", [T, C], F32, kind="ExternalOutput").ap()
    with tile.TileContext(nc) as tc:
        _emit(nc, tc, xT, wqk, wv, wp, out)
    nc.compile()
    _NC_CACHE = nc
    return nc


def make_in_maps(x, W_attn, W_proj):
    x = np.asarray(x, dtype=np.float32)
    W_attn = np.asarray(W_attn, dtype=np.float32)
    W_proj = np.asarray(W_proj, dtype=np.float32)
    in_maps = []
    for core in range(8):
        b = core // 2
        h0 = HPC * (core % 2)
        xT = np.ascontiguousarray(x[b].T).astype(np.float16)
        q_cols = W_attn[:, 64 * h0:64 * h0 + 384]
        k_cols = W_attn[:, 768 + 64 * h0:768 + 64 * h0 + 384]
        wqk = np.concatenate([q_cols, k_cols], axis=1).astype(np.float16)
        wv = W_attn[:, 1536 + 64 * h0:1536 + 64 * h0 + 384].astype(np.float16)
        wp = np.ascontiguousarray(W_proj[64 * h0:64 * h0 + 384, :])
        in_maps.append({"xT": xT, "wqk": np.ascontiguousarray(wqk),
                        "wv": wv, "wp": wp})
    return in_maps


def kernel(x, W_attn, W_proj, _trace=False, _trace_kwargs=None):
    nc = build_nc()
    in_maps = make_in_maps(x, W_attn, W_proj)
    res = run_bass_kernel_spmd(nc, in_maps, list(range(8)), trace=_trace,
                               **(_trace_kwargs or {}))
    outs = [res.results[c]["out"] for c in range(8)]
    y = np.stack([outs[2 * b] + outs[2 * b + 1] for b in range(4)]).astype(np.float32)
    if _trace:
        return y, res
    return y
